# revision 17
# baseline (speedup 1.0000x reference)
"""MobilityGNNLayer Trainium2 kernel (8 NeuronCores, SPMD, no collectives).

Sharding: 1D partition of the destination axis (columns of mobility_matrix).
Core c owns destination nodes i in [c*1024, (c+1)*1024).

Math (validated to rel 6.4e-3 vs the fp32 reference under the harness
metric):  the reference normalizes columns of M, thresholds at 1e-6,
aggregates the W_in-transformed features with a weighted mean, applies
W_out, residual, LN.  The threshold mask is numerically irrelevant
(validated), the column normalization cancels between numerator and weight
sum, and both W_in and W_out commute out of the aggregation because the
per-row 1/wsum scaling commutes with right-multiplication:
    agg_i @ Wc = (num_i / wsum_i) @ Wc = (sum_j M[j,i] * Xc[j,:]) / wsum_i
with Xc = X @ Wc precomputed on the host (Wc = W_in @ W_out).  So with
    G = M^T @ [Xc | 1 | 0]   (per-core [1024, 258] from its column shard)
    xrb = X[shard] + (b_in @ W_out + b_out)
    out_i = LN(G[i,:256] / G[i,256] + xrb_i) * ln_scale + ln_bias
No transpose and no second matmul on the device - the whole epilogue is
element-wise + LayerNorm.

Inputs stream in fp16 (halves HBM traffic; fp16 keeps 11 mantissa bits so
the quantization error lands ~6e-3 on the harness rel metric, vs the 2e-2
gate).  PSUM accumulates fp32.  Output is written fp16 and upcast on host.

Schedule: zone 1 (j-tiles 0..31) is j-supertile-major so the single paced
sync-queue DMA stream interleaves M chunks with the replicated Xc tiles;
zone 2 (j-tiles 32..63) is i-block-major so each destination block's PSUM
group closes early and its epilogue (recip/scale/add/LN on ACT+DVE+GPSIMD)
hides under the next block's matmuls.  All large DMAs are host-packed so
every transfer is one long contiguous run per SBUF partition.
"""

import numpy as np

import concourse.bass as bass
import concourse.mybir as mybir
import concourse.tile as tile
from concourse import bacc
from concourse.bass import ts
from concourse.bass_utils import run_bass_kernel_spmd

F32 = mybir.dt.float32
F16 = mybir.dt.float16
AF = mybir.ActivationFunctionType

N, D, NCORES = 8192, 256, 8
P = 128
LN_EPS = 1e-5

S1JT = 24            # zone-1 j-tiles (supertile-major)
Z2JT = 64 - S1JT     # zone-2 j-tiles (block-major)


def build_program(ln_affine=False):
    s = N // NCORES          # 1024 shard width (dest nodes per core)
    njt = N // P             # 64 contraction tiles
    nib = s // P             # 8 output row-blocks per core
    daug = D + 2             # [Xc | 1 | 0]
    nst1 = S1JT // 8         # zone-1 supertiles (8 j-tiles each)

    nc = bacc.Bacc("TRN2", target_bir_lowering=False, debug=False,
                   num_devices=NCORES)
    m_z1 = nc.dram_tensor("m_z1", [P, S1JT * s], F16, kind="ExternalInput")
    m_z2 = nc.dram_tensor("m_z2", [P, nib * Z2JT * P], F16,
                          kind="ExternalInput")
    x_aug = nc.dram_tensor("x_aug", [P, njt * daug], F16,
                           kind="ExternalInput")
    xrb_d = nc.dram_tensor("xrb", [P, nib * D], F32, kind="ExternalInput")
    ln_s = nc.dram_tensor("ln_s", [1, D], F32, kind="ExternalInput")
    ln_b = nc.dram_tensor("ln_b", [1, D], F32, kind="ExternalInput")
    out = nc.dram_tensor("out_shard", [P, nib * D], F16,
                         kind="ExternalOutput")

    with tile.TileContext(nc) as tc:
        with (
            tc.tile_pool(name="const", bufs=1) as const,
            tc.tile_pool(name="z1pool", bufs=3) as z1pool,
            tc.tile_pool(name="z2pool", bufs=4) as z2pool,
            tc.tile_pool(name="work", bufs=1) as work,
            tc.tile_pool(name="pp", bufs=1, space="PSUM") as pp,
        ):
            # small constants first (cheap; ACT Rsqrt table loads at t~0
            # instead of stalling the epilogue)
            eps_t = const.tile([P, 1], F32)
            nc.vector.memset(eps_t[:], LN_EPS)
            warm = const.tile([P, 2], F32)
            nc.scalar.activation(warm[:], eps_t[:].to_broadcast((P, 2)),
                                 AF.Sqrt, bias=eps_t[:], scale=1.0)
            if ln_affine:
                lns_bc = const.tile([P, D], F32)
                nc.scalar.dma_start(lns_bc[:], ln_s[:].to_broadcast((P, D)))
                lnb_bc = const.tile([P, D], F32)
                nc.scalar.dma_start(lnb_bc[:], ln_b[:].to_broadcast((P, D)))

            # ---- two paced DMA streams: M alone on the sync queue (fine
            # 4-j-tile chunks so the PE is fed from ~7us), Xc + xrb on the
            # scalar queue (own sequencer, so descriptor pushes of the two
            # streams do not serialize). ----
            xaug = const.tile([P, njt, daug], F16)
            xrb = const.tile([P, nib, D], F32)
            nc.scalar.dma_start(xaug[:, 0:1, :], x_aug[:, 0:daug])
            nc.scalar.dma_start(xaug[:, 1:8, :], x_aug[:, daug:8 * daug])
            for lo, hi in ((8, 16), (16, 32), (32, 48), (48, 64)):
                nc.scalar.dma_start(xaug[:, lo:hi, :],
                                    x_aug[:, lo * daug:hi * daug])

            z1t = [z1pool.tile([P, 8, s], F16, name="z1")
                   for st in range(nst1)]
            nc.sync.dma_start(z1t[0][:, 0:1, :], m_z1[:, 0:s])
            nc.sync.dma_start(z1t[0][:, 1:3, :], m_z1[:, s:3 * s])
            nc.sync.dma_start(z1t[0][:, 3:8, :], m_z1[:, 3 * s:8 * s])
            for st in range(1, nst1):
                base = st * 8 * s
                nc.sync.dma_start(z1t[st][:, 0:4, :],
                                  m_z1[:, base:base + 4 * s])
                nc.sync.dma_start(z1t[st][:, 4:8, :],
                                  m_z1[:, base + 4 * s:base + 8 * s])
            z2t = []
            for b in range(nib):
                t = z2pool.tile([P, Z2JT, P], F16, name="z2")
                z2t.append(t)
                nc.sync.dma_start(
                    t[:], m_z2[:, b * Z2JT * P:(b + 1) * Z2JT * P])
            # xrb (1 MB fp32) is only needed by the first epilogue (~47us);
            # gate it on zone-2 block 0's arrival so it stays out of the
            # zone-1 bandwidth crunch.
            gate = const.tile([P, 2], F32)
            nc.scalar.activation(gate[:], z2t[1][:, 0, 0:2], AF.Copy)
            nc.scalar.dma_start(xrb[:], xrb_d[:])

            # ---- matmuls: G[b] += M_tile^T @ Xc_aug[jt] ----
            g = [pp.tile([P, daug], F32, tag=f"g{b}", name=f"g{b}")
                 for b in range(nib)]
            for st in range(nst1):
                for t in range(8):
                    jt = st * 8 + t
                    for b in range(nib):
                        nc.tensor.matmul(g[b][:],
                                         lhsT=z1t[st][:, t, ts(b, P)],
                                         rhs=xaug[:, jt, :],
                                         start=(jt == 0), stop=False)
            for b in range(nib):
                for t in range(Z2JT):
                    nc.tensor.matmul(g[b][:], lhsT=z2t[b][:, t, :],
                                     rhs=xaug[:, S1JT + t, :],
                                     start=False, stop=(t == Z2JT - 1))

                # ---- per-block epilogue (hides under next block's MMs;
                # only the last block's chain is exec-time tail, so the
                # final two blocks stay off the slow GPSIMD adder) ----
                recip = work.tile([P, 1], F32, name=f"recip{b}")
                nc.vector.reciprocal(recip[:], g[b][:, D:D + 1])
                tt = work.tile([P, D], F32, name=f"t{b}")
                if b % 2 == 0:
                    nc.scalar.activation(tt[:], g[b][:, 0:D], AF.Copy,
                                         scale=recip[:])
                else:
                    nc.vector.tensor_scalar(tt[:], g[b][:, 0:D], recip[:],
                                            None, op0=mybir.AluOpType.mult)
                y = work.tile([P, D], F32, name=f"y{b}")
                if b < nib - 2:
                    nc.gpsimd.tensor_add(y[:], tt[:], xrb[:, b, :])
                else:
                    nc.vector.tensor_add(y[:], tt[:], xrb[:, b, :])
                st6 = work.tile([P, 6], F32, name=f"st6_{b}")
                nc.vector.bn_stats(st6[:], y[:])
                mv = work.tile([P, 2], F32, name=f"mv{b}")
                nc.vector.bn_aggr(mv[:], st6[:])
                # rstd = 1/sqrt(var + eps)
                stdv = work.tile([P, 1], F32, name=f"stdv{b}")
                nc.scalar.activation(stdv[:], mv[:, 1:2], AF.Sqrt,
                                     bias=eps_t[:], scale=1.0)
                rstd = work.tile([P, 1], F32, name=f"rstd{b}")
                nc.vector.reciprocal(rstd[:], stdv[:])
                bln = work.tile([P, 1], F32, name=f"bln{b}")
                nc.vector.scalar_tensor_tensor(
                    bln[:], in0=mv[:, 0:1], scalar=-1.0, in1=rstd[:],
                    op0=mybir.AluOpType.mult, op1=mybir.AluOpType.mult)

                yn = work.tile([P, D], F32 if ln_affine else F16,
                               name=f"yn{b}")
                if b % 2 == 0:   # split normalize across ACT and DVE
                    nc.scalar.activation(yn[:], y[:], AF.Identity,
                                         bias=bln[:], scale=rstd[:])
                else:
                    nc.vector.tensor_scalar(yn[:], y[:], rstd[:], bln[:],
                                            op0=mybir.AluOpType.mult,
                                            op1=mybir.AluOpType.add)
                res = yn
                if ln_affine:
                    t1 = work.tile([P, D], F32, name=f"aff{b}")
                    nc.vector.tensor_mul(t1[:], yn[:], lns_bc[:])
                    res = work.tile([P, D], F16, name=f"aff2_{b}")
                    nc.vector.tensor_add(res[:], t1[:], lnb_bc[:])
                nc.gpsimd.dma_start(out[:, b * D:(b + 1) * D], res[:])

    nc.compile()
    return nc


_cache = {}


def _get_program(ln_affine):
    if ln_affine not in _cache:
        _cache[ln_affine] = build_program(ln_affine=ln_affine)
    return _cache[ln_affine]


def _pack(a, blocks, row_len):
    """[blocks*128, row_len] -> [128, blocks*row_len] with logical row
    blk*128+p at (p, blk*row_len)."""
    return np.ascontiguousarray(
        a.reshape(blocks, P, row_len).transpose(1, 0, 2).reshape(
            P, blocks * row_len))


def prepare_inputs(node_features, mobility_matrix, W_in, b_in, W_out, b_out,
                   ln_scale, ln_bias):
    x = np.asarray(node_features, dtype=np.float32)
    m16 = np.asarray(mobility_matrix, dtype=np.float16)
    w_in = np.asarray(W_in, dtype=np.float64)
    b_in_ = np.asarray(b_in, dtype=np.float64)
    w_out = np.asarray(W_out, dtype=np.float64)
    b_out_ = np.asarray(b_out, dtype=np.float64)
    lns = np.asarray(ln_scale, dtype=np.float32)
    lnb = np.asarray(ln_bias, dtype=np.float32)

    w_c = (w_in @ w_out).astype(np.float32)
    bias_c = (b_in_ @ w_out + b_out_).astype(np.float32)

    s = N // NCORES
    ln_affine = not (np.all(lns == 1.0) and np.all(lnb == 0.0))

    xc = x @ w_c
    x_aug = np.zeros((N, D + 2), dtype=np.float16)
    x_aug[:, :D] = xc
    x_aug[:, D] = 1.0
    x_aug_p = _pack(x_aug, N // P, D + 2)

    in_maps = []
    for c in range(NCORES):
        msh = m16[:, c * s:(c + 1) * s]
        z1 = _pack(msh[0:S1JT * P], S1JT, s)
        z2 = np.ascontiguousarray(
            msh[S1JT * P:].reshape(Z2JT, P, s // P, P)
            .transpose(1, 2, 0, 3).reshape(P, (s // P) * Z2JT * P))
        in_maps.append({
            "m_z1": z1,
            "m_z2": z2,
            "x_aug": x_aug_p,
            "xrb": _pack(x[c * s:(c + 1) * s] + bias_c, s // P, D),
            "ln_s": lns.reshape(1, D),
            "ln_b": lnb.reshape(1, D),
        })
    return in_maps, ln_affine


def run(in_maps, ln_affine, **kwargs):
    nc = _get_program(ln_affine)
    return run_bass_kernel_spmd(nc, in_maps, core_ids=list(range(NCORES)),
                                **kwargs)


def unpack_output(res) -> np.ndarray:
    outs = []
    for c in range(NCORES):
        o = res.results[c]["out_shard"]
        outs.append(o.reshape(P, N // NCORES // P, D).transpose(1, 0, 2)
                    .reshape(N // NCORES, D).astype(np.float32))
    return np.concatenate(outs, axis=0)


def kernel(**inputs) -> np.ndarray:
    in_maps, ln_affine = prepare_inputs(**inputs)
    return unpack_output(run(in_maps, ln_affine))


# revision 18
# speedup vs baseline: 1.0086x; 1.0086x over previous
"""MobilityGNNLayer Trainium2 kernel (8 NeuronCores, SPMD, no collectives).

Sharding: 1D partition of the destination axis (columns of mobility_matrix).
Core c owns destination nodes i in [c*1024, (c+1)*1024).

Math (validated to rel 6.4e-3 vs the fp32 reference under the harness
metric):  the reference normalizes columns of M, thresholds at 1e-6,
aggregates the W_in-transformed features with a weighted mean, applies
W_out, residual, LN.  The threshold mask is numerically irrelevant
(validated), the column normalization cancels between numerator and weight
sum, and both W_in and W_out commute out of the aggregation because the
per-row 1/wsum scaling commutes with right-multiplication:
    agg_i @ Wc = (num_i / wsum_i) @ Wc = (sum_j M[j,i] * Xc[j,:]) / wsum_i
with Xc = X @ Wc precomputed on the host (Wc = W_in @ W_out).  So with
    G = M^T @ [Xc | 1 | 0]   (per-core [1024, 258] from its column shard)
    xrb = X[shard] + (b_in @ W_out + b_out)
    out_i = LN(G[i,:256] / G[i,256] + xrb_i) * ln_scale + ln_bias
No transpose and no second matmul on the device - the whole epilogue is
element-wise + LayerNorm.

Inputs stream in fp16 (halves HBM traffic; fp16 keeps 11 mantissa bits so
the quantization error lands ~6e-3 on the harness rel metric, vs the 2e-2
gate).  PSUM accumulates fp32.  Output is written fp16 and upcast on host.

Schedule: zone 1 (j-tiles 0..31) is j-supertile-major so the single paced
sync-queue DMA stream interleaves M chunks with the replicated Xc tiles;
zone 2 (j-tiles 32..63) is i-block-major so each destination block's PSUM
group closes early and its epilogue (recip/scale/add/LN on ACT+DVE+GPSIMD)
hides under the next block's matmuls.  All large DMAs are host-packed so
every transfer is one long contiguous run per SBUF partition.
"""

import numpy as np

import concourse.bass as bass
import concourse.mybir as mybir
import concourse.tile as tile
from concourse import bacc
from concourse.bass import ts
from concourse.bass_utils import run_bass_kernel_spmd

F32 = mybir.dt.float32
F16 = mybir.dt.float16
AF = mybir.ActivationFunctionType

N, D, NCORES = 8192, 256, 8
P = 128
LN_EPS = 1e-5

S1JT = 32            # zone-1 j-tiles (supertile-major)
Z2JT = 64 - S1JT     # zone-2 j-tiles (block-major)


def build_program(ln_affine=False):
    s = N // NCORES          # 1024 shard width (dest nodes per core)
    njt = N // P             # 64 contraction tiles
    nib = s // P             # 8 output row-blocks per core
    daug = D + 2             # [Xc | 1 | 0]
    nst1 = S1JT // 8         # zone-1 supertiles (8 j-tiles each)

    nc = bacc.Bacc("TRN2", target_bir_lowering=False, debug=False,
                   num_devices=NCORES)
    m_z1 = nc.dram_tensor("m_z1", [P, S1JT * s], F16, kind="ExternalInput")
    m_z2 = nc.dram_tensor("m_z2", [P, nib * Z2JT * P], F16,
                          kind="ExternalInput")
    x_aug = nc.dram_tensor("x_aug", [P, njt * daug], F16,
                           kind="ExternalInput")
    xrb_d = nc.dram_tensor("xrb", [P, nib * D], F32, kind="ExternalInput")
    ln_s = nc.dram_tensor("ln_s", [1, D], F32, kind="ExternalInput")
    ln_b = nc.dram_tensor("ln_b", [1, D], F32, kind="ExternalInput")
    out = nc.dram_tensor("out_shard", [P, nib * D], F16,
                         kind="ExternalOutput")

    with tile.TileContext(nc) as tc:
        with (
            tc.tile_pool(name="const", bufs=1) as const,
            tc.tile_pool(name="z1pool", bufs=3) as z1pool,
            tc.tile_pool(name="z2pool", bufs=4) as z2pool,
            tc.tile_pool(name="work", bufs=1) as work,
            tc.tile_pool(name="pp", bufs=1, space="PSUM") as pp,
        ):
            # small constants first (cheap; ACT Rsqrt table loads at t~0
            # instead of stalling the epilogue)
            eps_t = const.tile([P, 1], F32)
            nc.vector.memset(eps_t[:], LN_EPS)
            warm = const.tile([P, 2], F32)
            nc.scalar.activation(warm[:], eps_t[:].to_broadcast((P, 2)),
                                 AF.Sqrt, bias=eps_t[:], scale=1.0)
            if ln_affine:
                lns_bc = const.tile([P, D], F32)
                nc.scalar.dma_start(lns_bc[:], ln_s[:].to_broadcast((P, D)))
                lnb_bc = const.tile([P, D], F32)
                nc.scalar.dma_start(lnb_bc[:], ln_b[:].to_broadcast((P, D)))

            # ---- two paced DMA streams: M alone on the sync queue (fine
            # 4-j-tile chunks so the PE is fed from ~7us), Xc + xrb on the
            # scalar queue (own sequencer, so descriptor pushes of the two
            # streams do not serialize). ----
            xaug = const.tile([P, njt, daug], F16)
            xrb = const.tile([P, nib, D], F32)
            nc.scalar.dma_start(xaug[:, 0:1, :], x_aug[:, 0:daug])
            nc.scalar.dma_start(xaug[:, 1:8, :], x_aug[:, daug:8 * daug])
            for lo, hi in ((8, 16), (16, 32), (32, 48), (48, 64)):
                nc.scalar.dma_start(xaug[:, lo:hi, :],
                                    x_aug[:, lo * daug:hi * daug])

            z1t = [z1pool.tile([P, 8, s], F16, name="z1")
                   for st in range(nst1)]
            nc.sync.dma_start(z1t[0][:, 0:1, :], m_z1[:, 0:s])
            nc.sync.dma_start(z1t[0][:, 1:3, :], m_z1[:, s:3 * s])
            nc.sync.dma_start(z1t[0][:, 3:8, :], m_z1[:, 3 * s:8 * s])
            for st in range(1, nst1):
                base = st * 8 * s
                nc.sync.dma_start(z1t[st][:, 0:4, :],
                                  m_z1[:, base:base + 4 * s])
                nc.sync.dma_start(z1t[st][:, 4:8, :],
                                  m_z1[:, base + 4 * s:base + 8 * s])
            z2t = []
            for b in range(nib):
                t = z2pool.tile([P, Z2JT, P], F16, name="z2")
                z2t.append(t)
                nc.sync.dma_start(
                    t[:], m_z2[:, b * Z2JT * P:(b + 1) * Z2JT * P])
            # xrb (1 MB fp32) is only needed by the first epilogue (~47us);
            # gate it on zone-2 block 0's arrival so it stays out of the
            # zone-1 bandwidth crunch.
            gate = const.tile([P, 2], F32)
            nc.scalar.activation(gate[:], z2t[1][:, 0, 0:2], AF.Copy)
            nc.scalar.dma_start(xrb[:], xrb_d[:])

            # ---- matmuls: G[b] += M_tile^T @ Xc_aug[jt] ----
            g = [pp.tile([P, daug], F32, tag=f"g{b}", name=f"g{b}")
                 for b in range(nib)]
            for st in range(nst1):
                for t in range(8):
                    jt = st * 8 + t
                    for b in range(nib):
                        nc.tensor.matmul(g[b][:],
                                         lhsT=z1t[st][:, t, ts(b, P)],
                                         rhs=xaug[:, jt, :],
                                         start=(jt == 0), stop=False)
            for b in range(nib):
                for t in range(Z2JT):
                    nc.tensor.matmul(g[b][:], lhsT=z2t[b][:, t, :],
                                     rhs=xaug[:, S1JT + t, :],
                                     start=False, stop=(t == Z2JT - 1))

                # ---- per-block epilogue (hides under next block's MMs;
                # only the last block's chain is exec-time tail, so the
                # final two blocks stay off the slow GPSIMD adder) ----
                recip = work.tile([P, 1], F32, name=f"recip{b}")
                nc.vector.reciprocal(recip[:], g[b][:, D:D + 1])
                tt = work.tile([P, D], F32, name=f"t{b}")
                if b % 2 == 0:
                    nc.scalar.activation(tt[:], g[b][:, 0:D], AF.Copy,
                                         scale=recip[:])
                else:
                    nc.vector.tensor_scalar(tt[:], g[b][:, 0:D], recip[:],
                                            None, op0=mybir.AluOpType.mult)
                y = work.tile([P, D], F32, name=f"y{b}")
                if b < nib - 2:
                    nc.gpsimd.tensor_add(y[:], tt[:], xrb[:, b, :])
                else:
                    nc.vector.tensor_add(y[:], tt[:], xrb[:, b, :])
                st6 = work.tile([P, 6], F32, name=f"st6_{b}")
                nc.vector.bn_stats(st6[:], y[:])
                mv = work.tile([P, 2], F32, name=f"mv{b}")
                nc.vector.bn_aggr(mv[:], st6[:])
                # rstd = 1/sqrt(var + eps)
                stdv = work.tile([P, 1], F32, name=f"stdv{b}")
                nc.scalar.activation(stdv[:], mv[:, 1:2], AF.Sqrt,
                                     bias=eps_t[:], scale=1.0)
                rstd = work.tile([P, 1], F32, name=f"rstd{b}")
                nc.vector.reciprocal(rstd[:], stdv[:])
                bln = work.tile([P, 1], F32, name=f"bln{b}")
                nc.vector.scalar_tensor_tensor(
                    bln[:], in0=mv[:, 0:1], scalar=-1.0, in1=rstd[:],
                    op0=mybir.AluOpType.mult, op1=mybir.AluOpType.mult)

                yn = work.tile([P, D], F32 if ln_affine else F16,
                               name=f"yn{b}")
                if b % 2 == 0:   # split normalize across ACT and DVE
                    nc.scalar.activation(yn[:], y[:], AF.Identity,
                                         bias=bln[:], scale=rstd[:])
                else:
                    nc.vector.tensor_scalar(yn[:], y[:], rstd[:], bln[:],
                                            op0=mybir.AluOpType.mult,
                                            op1=mybir.AluOpType.add)
                res = yn
                if ln_affine:
                    t1 = work.tile([P, D], F32, name=f"aff{b}")
                    nc.vector.tensor_mul(t1[:], yn[:], lns_bc[:])
                    res = work.tile([P, D], F16, name=f"aff2_{b}")
                    nc.vector.tensor_add(res[:], t1[:], lnb_bc[:])
                nc.gpsimd.dma_start(out[:, b * D:(b + 1) * D], res[:])

    nc.compile()
    return nc


_cache = {}


def _get_program(ln_affine):
    if ln_affine not in _cache:
        _cache[ln_affine] = build_program(ln_affine=ln_affine)
    return _cache[ln_affine]


def _pack(a, blocks, row_len):
    """[blocks*128, row_len] -> [128, blocks*row_len] with logical row
    blk*128+p at (p, blk*row_len)."""
    return np.ascontiguousarray(
        a.reshape(blocks, P, row_len).transpose(1, 0, 2).reshape(
            P, blocks * row_len))


def prepare_inputs(node_features, mobility_matrix, W_in, b_in, W_out, b_out,
                   ln_scale, ln_bias):
    x = np.asarray(node_features, dtype=np.float32)
    m16 = np.asarray(mobility_matrix, dtype=np.float16)
    w_in = np.asarray(W_in, dtype=np.float64)
    b_in_ = np.asarray(b_in, dtype=np.float64)
    w_out = np.asarray(W_out, dtype=np.float64)
    b_out_ = np.asarray(b_out, dtype=np.float64)
    lns = np.asarray(ln_scale, dtype=np.float32)
    lnb = np.asarray(ln_bias, dtype=np.float32)

    w_c = (w_in @ w_out).astype(np.float32)
    bias_c = (b_in_ @ w_out + b_out_).astype(np.float32)

    s = N // NCORES
    ln_affine = not (np.all(lns == 1.0) and np.all(lnb == 0.0))

    xc = x @ w_c
    x_aug = np.zeros((N, D + 2), dtype=np.float16)
    x_aug[:, :D] = xc
    x_aug[:, D] = 1.0
    x_aug_p = _pack(x_aug, N // P, D + 2)

    in_maps = []
    for c in range(NCORES):
        msh = m16[:, c * s:(c + 1) * s]
        z1 = _pack(msh[0:S1JT * P], S1JT, s)
        z2 = np.ascontiguousarray(
            msh[S1JT * P:].reshape(Z2JT, P, s // P, P)
            .transpose(1, 2, 0, 3).reshape(P, (s // P) * Z2JT * P))
        in_maps.append({
            "m_z1": z1,
            "m_z2": z2,
            "x_aug": x_aug_p,
            "xrb": _pack(x[c * s:(c + 1) * s] + bias_c, s // P, D),
            "ln_s": lns.reshape(1, D),
            "ln_b": lnb.reshape(1, D),
        })
    return in_maps, ln_affine


def run(in_maps, ln_affine, **kwargs):
    nc = _get_program(ln_affine)
    return run_bass_kernel_spmd(nc, in_maps, core_ids=list(range(NCORES)),
                                **kwargs)


def unpack_output(res) -> np.ndarray:
    outs = []
    for c in range(NCORES):
        o = res.results[c]["out_shard"]
        outs.append(o.reshape(P, N // NCORES // P, D).transpose(1, 0, 2)
                    .reshape(N // NCORES, D).astype(np.float32))
    return np.concatenate(outs, axis=0)


def kernel(**inputs) -> np.ndarray:
    in_maps, ln_affine = prepare_inputs(**inputs)
    return unpack_output(run(in_maps, ln_affine))


# revision 20
# speedup vs baseline: 1.0144x; 1.0057x over previous
"""MobilityGNNLayer Trainium2 kernel (8 NeuronCores, SPMD, no collectives).

Sharding: 1D partition of the destination axis (columns of mobility_matrix).
Core c owns destination nodes i in [c*1024, (c+1)*1024).

Math (validated to rel 6.4e-3 vs the fp32 reference under the harness
metric):  the reference normalizes columns of M, thresholds at 1e-6,
aggregates the W_in-transformed features with a weighted mean, applies
W_out, residual, LN.  The threshold mask is numerically irrelevant
(validated), the column normalization cancels between numerator and weight
sum, and both W_in and W_out commute out of the aggregation because the
per-row 1/wsum scaling commutes with right-multiplication:
    agg_i @ Wc = (num_i / wsum_i) @ Wc = (sum_j M[j,i] * Xc[j,:]) / wsum_i
with Xc = X @ Wc precomputed on the host (Wc = W_in @ W_out).  So with
    G = M^T @ [Xc | 1 | 0]   (per-core [1024, 258] from its column shard)
    xrb = X[shard] + (b_in @ W_out + b_out)
    out_i = LN(G[i,:256] / G[i,256] + xrb_i) * ln_scale + ln_bias
No transpose and no second matmul on the device - the whole epilogue is
element-wise + LayerNorm.

Inputs stream in fp16 (halves HBM traffic; fp16 keeps 11 mantissa bits so
the quantization error lands ~6e-3 on the harness rel metric, vs the 2e-2
gate).  PSUM accumulates fp32.  Output is written fp16 and upcast on host.

Schedule: zone 1 (j-tiles 0..31) is j-supertile-major so the single paced
sync-queue DMA stream interleaves M chunks with the replicated Xc tiles;
zone 2 (j-tiles 32..63) is i-block-major so each destination block's PSUM
group closes early and its epilogue (recip/scale/add/LN on ACT+DVE+GPSIMD)
hides under the next block's matmuls.  All large DMAs are host-packed so
every transfer is one long contiguous run per SBUF partition.
"""

import numpy as np

import concourse.bass as bass
import concourse.mybir as mybir
import concourse.tile as tile
from concourse import bacc
from concourse.bass import ts
from concourse.bass_utils import run_bass_kernel_spmd

F32 = mybir.dt.float32
F16 = mybir.dt.float16
AF = mybir.ActivationFunctionType

N, D, NCORES = 8192, 256, 8
P = 128
LN_EPS = 1e-5

S1JT = 32            # zone-1 j-tiles (supertile-major)
Z2JT = 64 - S1JT     # zone-2 j-tiles (block-major)


def build_program(ln_affine=False):
    s = N // NCORES          # 1024 shard width (dest nodes per core)
    njt = N // P             # 64 contraction tiles
    nib = s // P             # 8 output row-blocks per core
    daug = D + 2             # [Xc | 1 | 0]
    nst1 = S1JT // 8         # zone-1 supertiles (8 j-tiles each)

    nc = bacc.Bacc("TRN2", target_bir_lowering=False, debug=False,
                   num_devices=NCORES)
    m_z1 = nc.dram_tensor("m_z1", [P, S1JT * s], F16, kind="ExternalInput")
    m_z2 = nc.dram_tensor("m_z2", [P, nib * Z2JT * P], F16,
                          kind="ExternalInput")
    x_aug = nc.dram_tensor("x_aug", [P, njt * daug], F16,
                           kind="ExternalInput")
    xrb_d = nc.dram_tensor("xrb", [P, nib * D], F32, kind="ExternalInput")
    ln_s = nc.dram_tensor("ln_s", [1, D], F32, kind="ExternalInput")
    ln_b = nc.dram_tensor("ln_b", [1, D], F32, kind="ExternalInput")
    out = nc.dram_tensor("out_shard", [P, nib * D], F16,
                         kind="ExternalOutput")

    with tile.TileContext(nc) as tc:
        with (
            tc.tile_pool(name="const", bufs=1) as const,
            tc.tile_pool(name="z1pool", bufs=3) as z1pool,
            tc.tile_pool(name="z2pool", bufs=4) as z2pool,
            tc.tile_pool(name="work", bufs=1) as work,
            tc.tile_pool(name="pp", bufs=1, space="PSUM") as pp,
        ):
            # small constants first (cheap; ACT Rsqrt table loads at t~0
            # instead of stalling the epilogue)
            eps_t = const.tile([P, 1], F32)
            nc.vector.memset(eps_t[:], LN_EPS)
            warm = const.tile([P, 2], F32)
            nc.scalar.activation(warm[:], eps_t[:].to_broadcast((P, 2)),
                                 AF.Sqrt, bias=eps_t[:], scale=1.0)
            if ln_affine:
                lns_bc = const.tile([P, D], F32)
                nc.scalar.dma_start(lns_bc[:], ln_s[:].to_broadcast((P, D)))
                lnb_bc = const.tile([P, D], F32)
                nc.scalar.dma_start(lnb_bc[:], ln_b[:].to_broadcast((P, D)))

            # ---- two paced DMA streams: M alone on the sync queue (fine
            # 4-j-tile chunks so the PE is fed from ~7us), Xc + xrb on the
            # scalar queue (own sequencer, so descriptor pushes of the two
            # streams do not serialize). ----
            xaug = const.tile([P, njt, daug], F16)
            xrb = const.tile([P, nib, D], F32)
            nc.scalar.dma_start(xaug[:, 0:1, :], x_aug[:, 0:daug])
            nc.scalar.dma_start(xaug[:, 1:8, :], x_aug[:, daug:8 * daug])
            for lo, hi in ((8, 16), (16, 32), (32, 48), (48, 64)):
                nc.scalar.dma_start(xaug[:, lo:hi, :],
                                    x_aug[:, lo * daug:hi * daug])

            z1t = [z1pool.tile([P, 8, s], F16, name="z1")
                   for st in range(nst1)]
            nc.sync.dma_start(z1t[0][:, 0:1, :], m_z1[:, 0:s])
            nc.sync.dma_start(z1t[0][:, 1:3, :], m_z1[:, s:3 * s])
            nc.sync.dma_start(z1t[0][:, 3:8, :], m_z1[:, 3 * s:8 * s])
            for st in range(1, nst1):
                base = st * 8 * s
                nc.sync.dma_start(z1t[st][:, 0:4, :],
                                  m_z1[:, base:base + 4 * s])
                nc.sync.dma_start(z1t[st][:, 4:8, :],
                                  m_z1[:, base + 4 * s:base + 8 * s])
            z2t = []
            for b in range(nib):
                t = z2pool.tile([P, Z2JT, P], F16, name="z2")
                z2t.append(t)
                nc.sync.dma_start(
                    t[:], m_z2[:, b * Z2JT * P:(b + 1) * Z2JT * P])
            # xrb (1 MB fp32) is only needed by the first epilogue (~47us);
            # gate it on zone-2 block 0's arrival so it stays out of the
            # zone-1 bandwidth crunch.
            gate = const.tile([P, 2], F32)
            nc.scalar.activation(gate[:], z2t[1][:, 0, 0:2], AF.Copy)
            nc.scalar.dma_start(xrb[:], xrb_d[:])

            # ---- matmuls: G[b] += M_tile^T @ Xc_aug[jt] ----
            g = [pp.tile([P, daug], F32, tag=f"g{b}", name=f"g{b}")
                 for b in range(nib)]
            for st in range(nst1):
                for t in range(8):
                    jt = st * 8 + t
                    for b in range(nib):
                        nc.tensor.matmul(g[b][:],
                                         lhsT=z1t[st][:, t, ts(b, P)],
                                         rhs=xaug[:, jt, :],
                                         start=(jt == 0), stop=False)
            for b in range(nib):
                for t in range(Z2JT):
                    nc.tensor.matmul(g[b][:], lhsT=z2t[b][:, t, :],
                                     rhs=xaug[:, S1JT + t, :],
                                     start=False, stop=(t == Z2JT - 1))

                # ---- per-block epilogue (hides under next block's MMs) --
                # LayerNorm is invariant to positive per-row scaling, so
                # instead of y = G/wsum + xrb we normalize
                # y2 = G + wsum*xrb directly - no reciprocal, no PSUM
                # evacuation pass.
                tt = work.tile([P, D], F32, name=f"t{b}")
                nc.vector.tensor_scalar(tt[:], xrb[:, b, :],
                                        g[b][:, D:D + 1], None,
                                        op0=mybir.AluOpType.mult)
                y = work.tile([P, D], F32, name=f"y{b}")
                nc.vector.tensor_add(y[:], tt[:], g[b][:, 0:D])
                st6 = work.tile([P, 6], F32, name=f"st6_{b}")
                nc.vector.bn_stats(st6[:], y[:])
                mv = work.tile([P, 2], F32, name=f"mv{b}")
                nc.vector.bn_aggr(mv[:], st6[:])
                # rstd = 1/sqrt(var + eps)
                stdv = work.tile([P, 1], F32, name=f"stdv{b}")
                nc.scalar.activation(stdv[:], mv[:, 1:2], AF.Sqrt,
                                     bias=eps_t[:], scale=1.0)
                rstd = work.tile([P, 1], F32, name=f"rstd{b}")
                nc.vector.reciprocal(rstd[:], stdv[:])
                bln = work.tile([P, 1], F32, name=f"bln{b}")
                nc.vector.scalar_tensor_tensor(
                    bln[:], in0=mv[:, 0:1], scalar=-1.0, in1=rstd[:],
                    op0=mybir.AluOpType.mult, op1=mybir.AluOpType.mult)

                yn = work.tile([P, D], F32 if ln_affine else F16,
                               name=f"yn{b}")
                if b % 2 == 0:   # split normalize across ACT and DVE
                    nc.scalar.activation(yn[:], y[:], AF.Identity,
                                         bias=bln[:], scale=rstd[:])
                else:
                    nc.vector.tensor_scalar(yn[:], y[:], rstd[:], bln[:],
                                            op0=mybir.AluOpType.mult,
                                            op1=mybir.AluOpType.add)
                res = yn
                if ln_affine:
                    t1 = work.tile([P, D], F32, name=f"aff{b}")
                    nc.vector.tensor_mul(t1[:], yn[:], lns_bc[:])
                    res = work.tile([P, D], F16, name=f"aff2_{b}")
                    nc.vector.tensor_add(res[:], t1[:], lnb_bc[:])
                nc.gpsimd.dma_start(out[:, b * D:(b + 1) * D], res[:])

    nc.compile()
    return nc


_cache = {}


def _get_program(ln_affine):
    if ln_affine not in _cache:
        _cache[ln_affine] = build_program(ln_affine=ln_affine)
    return _cache[ln_affine]


def _pack(a, blocks, row_len):
    """[blocks*128, row_len] -> [128, blocks*row_len] with logical row
    blk*128+p at (p, blk*row_len)."""
    return np.ascontiguousarray(
        a.reshape(blocks, P, row_len).transpose(1, 0, 2).reshape(
            P, blocks * row_len))


def prepare_inputs(node_features, mobility_matrix, W_in, b_in, W_out, b_out,
                   ln_scale, ln_bias):
    x = np.asarray(node_features, dtype=np.float32)
    m16 = np.asarray(mobility_matrix, dtype=np.float16)
    w_in = np.asarray(W_in, dtype=np.float64)
    b_in_ = np.asarray(b_in, dtype=np.float64)
    w_out = np.asarray(W_out, dtype=np.float64)
    b_out_ = np.asarray(b_out, dtype=np.float64)
    lns = np.asarray(ln_scale, dtype=np.float32)
    lnb = np.asarray(ln_bias, dtype=np.float32)

    w_c = (w_in @ w_out).astype(np.float32)
    bias_c = (b_in_ @ w_out + b_out_).astype(np.float32)

    s = N // NCORES
    ln_affine = not (np.all(lns == 1.0) and np.all(lnb == 0.0))

    xc = x @ w_c
    x_aug = np.zeros((N, D + 2), dtype=np.float16)
    x_aug[:, :D] = xc
    x_aug[:, D] = 1.0
    x_aug_p = _pack(x_aug, N // P, D + 2)

    in_maps = []
    for c in range(NCORES):
        msh = m16[:, c * s:(c + 1) * s]
        z1 = _pack(msh[0:S1JT * P], S1JT, s)
        z2 = np.ascontiguousarray(
            msh[S1JT * P:].reshape(Z2JT, P, s // P, P)
            .transpose(1, 2, 0, 3).reshape(P, (s // P) * Z2JT * P))
        in_maps.append({
            "m_z1": z1,
            "m_z2": z2,
            "x_aug": x_aug_p,
            "xrb": _pack(x[c * s:(c + 1) * s] + bias_c, s // P, D),
            "ln_s": lns.reshape(1, D),
            "ln_b": lnb.reshape(1, D),
        })
    return in_maps, ln_affine


def run(in_maps, ln_affine, **kwargs):
    nc = _get_program(ln_affine)
    return run_bass_kernel_spmd(nc, in_maps, core_ids=list(range(NCORES)),
                                **kwargs)


def unpack_output(res) -> np.ndarray:
    outs = []
    for c in range(NCORES):
        o = res.results[c]["out_shard"]
        outs.append(o.reshape(P, N // NCORES // P, D).transpose(1, 0, 2)
                    .reshape(N // NCORES, D).astype(np.float32))
    return np.concatenate(outs, axis=0)


def kernel(**inputs) -> np.ndarray:
    in_maps, ln_affine = prepare_inputs(**inputs)
    return unpack_output(run(in_maps, ln_affine))


# revision 21
# speedup vs baseline: 1.0479x; 1.0331x over previous
"""MobilityGNNLayer Trainium2 kernel (8 NeuronCores, SPMD, no collectives).

Sharding: 1D partition of the destination axis (columns of mobility_matrix).
Core c owns destination nodes i in [c*1024, (c+1)*1024).

Math (validated to rel 6.4e-3 vs the fp32 reference under the harness
metric):  the reference normalizes columns of M, thresholds at 1e-6,
aggregates the W_in-transformed features with a weighted mean, applies
W_out, residual, LN.  The threshold mask is numerically irrelevant
(validated), the column normalization cancels between numerator and weight
sum, and both W_in and W_out commute out of the aggregation because the
per-row 1/wsum scaling commutes with right-multiplication:
    agg_i @ Wc = (num_i / wsum_i) @ Wc = (sum_j M[j,i] * Xc[j,:]) / wsum_i
with Xc = X @ Wc precomputed on the host (Wc = W_in @ W_out).  So with
    G = M^T @ [Xc | 1 | 0]   (per-core [1024, 258] from its column shard)
    xrb = X[shard] + (b_in @ W_out + b_out)
    out_i = LN(G[i,:256] / G[i,256] + xrb_i) * ln_scale + ln_bias
No transpose and no second matmul on the device - the whole epilogue is
element-wise + LayerNorm.

Inputs stream in fp16 (halves HBM traffic; fp16 keeps 11 mantissa bits so
the quantization error lands ~6e-3 on the harness rel metric, vs the 2e-2
gate).  PSUM accumulates fp32.  Output is written fp16 and upcast on host.

Schedule: zone 1 (j-tiles 0..31) is j-supertile-major so the single paced
sync-queue DMA stream interleaves M chunks with the replicated Xc tiles;
zone 2 (j-tiles 32..63) is i-block-major so each destination block's PSUM
group closes early and its epilogue (recip/scale/add/LN on ACT+DVE+GPSIMD)
hides under the next block's matmuls.  All large DMAs are host-packed so
every transfer is one long contiguous run per SBUF partition.
"""

import numpy as np

import concourse.bass as bass
import concourse.mybir as mybir
import concourse.tile as tile
from concourse import bacc
from concourse.bass import ts
from concourse.bass_utils import run_bass_kernel_spmd

F32 = mybir.dt.float32
F16 = mybir.dt.float16
AF = mybir.ActivationFunctionType

N, D, NCORES = 8192, 256, 8
P = 128
LN_EPS = 1e-5

S1JT = 32            # zone-1 j-tiles (supertile-major)
Z2JT = 64 - S1JT     # zone-2 j-tiles (block-major)


def build_program(ln_affine=False):
    s = N // NCORES          # 1024 shard width (dest nodes per core)
    njt = N // P             # 64 contraction tiles
    nib = s // P             # 8 output row-blocks per core
    daug = D + 2             # [Xc | 1 | 0]
    nst1 = S1JT // 8         # zone-1 supertiles (8 j-tiles each)

    nc = bacc.Bacc("TRN2", target_bir_lowering=False, debug=False,
                   num_devices=NCORES)
    m_z1 = nc.dram_tensor("m_z1", [P, S1JT * s], F16, kind="ExternalInput")
    m_z2 = nc.dram_tensor("m_z2", [P, nib * Z2JT * P], F16,
                          kind="ExternalInput")
    x_aug = nc.dram_tensor("x_aug", [P, njt * daug], F16,
                           kind="ExternalInput")
    xrb_d = nc.dram_tensor("xrb", [P, nib * D], F32, kind="ExternalInput")
    ln_s = nc.dram_tensor("ln_s", [1, D], F32, kind="ExternalInput")
    ln_b = nc.dram_tensor("ln_b", [1, D], F32, kind="ExternalInput")
    out = nc.dram_tensor("out_shard", [P, nib * D], F16,
                         kind="ExternalOutput")

    with tile.TileContext(nc) as tc:
        with (
            tc.tile_pool(name="const", bufs=1) as const,
            tc.tile_pool(name="z1pool", bufs=3) as z1pool,
            tc.tile_pool(name="z2pool", bufs=4) as z2pool,
            tc.tile_pool(name="work", bufs=1) as work,
            tc.tile_pool(name="pp", bufs=1, space="PSUM") as pp,
        ):
            # small constants first (cheap; ACT Rsqrt table loads at t~0
            # instead of stalling the epilogue)
            eps_t = const.tile([P, 1], F32)
            nc.vector.memset(eps_t[:], LN_EPS)
            warm = const.tile([P, 2], F32)
            nc.scalar.activation(warm[:], eps_t[:].to_broadcast((P, 2)),
                                 AF.Sqrt, bias=eps_t[:], scale=1.0)
            if ln_affine:
                lns_bc = const.tile([P, D], F32)
                nc.scalar.dma_start(lns_bc[:], ln_s[:].to_broadcast((P, D)))
                lnb_bc = const.tile([P, D], F32)
                nc.scalar.dma_start(lnb_bc[:], ln_b[:].to_broadcast((P, D)))

            # ---- one paced DMA stream on the sync queue: a single
            # sequential HBM stream per core sustains higher bandwidth than
            # two competing queues (~425 vs ~370 GB/s measured). Xc tiles
            # interleave just-in-time ahead of their M chunks; chunks are
            # fine early so the PE starts ~7us in and never starves. ----
            xaug = const.tile([P, njt, daug], F16)
            xrb = const.tile([P, nib, D], F32)
            z1t = [z1pool.tile([P, 8, s], F16, name="z1")
                   for st in range(nst1)]
            nc.sync.dma_start(xaug[:, 0:1, :], x_aug[:, 0:daug])
            nc.sync.dma_start(z1t[0][:, 0:1, :], m_z1[:, 0:s])
            nc.sync.dma_start(xaug[:, 1:4, :], x_aug[:, daug:4 * daug])
            nc.sync.dma_start(z1t[0][:, 1:3, :], m_z1[:, s:3 * s])
            nc.sync.dma_start(xaug[:, 4:8, :], x_aug[:, 4 * daug:8 * daug])
            nc.sync.dma_start(z1t[0][:, 3:8, :], m_z1[:, 3 * s:8 * s])
            for st in range(1, nst1):
                base = st * 8 * s
                nc.sync.dma_start(xaug[:, 8 * st:8 * (st + 1), :],
                                  x_aug[:, 8 * st * daug:8 * (st + 1) * daug])
                nc.sync.dma_start(z1t[st][:, 0:4, :],
                                  m_z1[:, base:base + 4 * s])
                nc.sync.dma_start(z1t[st][:, 4:8, :],
                                  m_z1[:, base + 4 * s:base + 8 * s])
            nc.sync.dma_start(xaug[:, S1JT:48, :],
                              x_aug[:, S1JT * daug:48 * daug])
            z2t = []
            for b in range(nib):
                t = z2pool.tile([P, Z2JT, P], F16, name="z2")
                z2t.append(t)
                nc.sync.dma_start(
                    t[:], m_z2[:, b * Z2JT * P:(b + 1) * Z2JT * P])
                if b == 0:
                    nc.sync.dma_start(xaug[:, 48:64, :],
                                      x_aug[:, 48 * daug:64 * daug])
            # xrb (1 MB fp32) is only needed by the first epilogue (~47us);
            # ship it on the scalar queue, gated on zone-2 block 1's
            # arrival, so it stays out of the zone-1 bandwidth crunch.
            gate = const.tile([P, 2], F32)
            nc.scalar.activation(gate[:], z2t[1][:, 0, 0:2], AF.Copy)
            nc.scalar.dma_start(xrb[:], xrb_d[:])

            # ---- matmuls: G[b] += M_tile^T @ Xc_aug[jt] ----
            g = [pp.tile([P, daug], F32, tag=f"g{b}", name=f"g{b}")
                 for b in range(nib)]
            for st in range(nst1):
                for t in range(8):
                    jt = st * 8 + t
                    for b in range(nib):
                        nc.tensor.matmul(g[b][:],
                                         lhsT=z1t[st][:, t, ts(b, P)],
                                         rhs=xaug[:, jt, :],
                                         start=(jt == 0), stop=False)
            for b in range(nib):
                for t in range(Z2JT):
                    nc.tensor.matmul(g[b][:], lhsT=z2t[b][:, t, :],
                                     rhs=xaug[:, S1JT + t, :],
                                     start=False, stop=(t == Z2JT - 1))

                # ---- per-block epilogue (hides under next block's MMs) --
                # LayerNorm is invariant to positive per-row scaling, so
                # instead of y = G/wsum + xrb we normalize
                # y2 = G + wsum*xrb directly - no reciprocal, no PSUM
                # evacuation pass.
                tt = work.tile([P, D], F32, name=f"t{b}")
                nc.vector.tensor_scalar(tt[:], xrb[:, b, :],
                                        g[b][:, D:D + 1], None,
                                        op0=mybir.AluOpType.mult)
                y = work.tile([P, D], F32, name=f"y{b}")
                nc.vector.tensor_add(y[:], tt[:], g[b][:, 0:D])
                st6 = work.tile([P, 6], F32, name=f"st6_{b}")
                nc.vector.bn_stats(st6[:], y[:])
                mv = work.tile([P, 2], F32, name=f"mv{b}")
                nc.vector.bn_aggr(mv[:], st6[:])
                # rstd = 1/sqrt(var + eps)
                stdv = work.tile([P, 1], F32, name=f"stdv{b}")
                nc.scalar.activation(stdv[:], mv[:, 1:2], AF.Sqrt,
                                     bias=eps_t[:], scale=1.0)
                rstd = work.tile([P, 1], F32, name=f"rstd{b}")
                nc.vector.reciprocal(rstd[:], stdv[:])
                bln = work.tile([P, 1], F32, name=f"bln{b}")
                nc.vector.scalar_tensor_tensor(
                    bln[:], in0=mv[:, 0:1], scalar=-1.0, in1=rstd[:],
                    op0=mybir.AluOpType.mult, op1=mybir.AluOpType.mult)

                yn = work.tile([P, D], F32 if ln_affine else F16,
                               name=f"yn{b}")
                if b % 2 == 0:   # split normalize across ACT and DVE
                    nc.scalar.activation(yn[:], y[:], AF.Identity,
                                         bias=bln[:], scale=rstd[:])
                else:
                    nc.vector.tensor_scalar(yn[:], y[:], rstd[:], bln[:],
                                            op0=mybir.AluOpType.mult,
                                            op1=mybir.AluOpType.add)
                res = yn
                if ln_affine:
                    t1 = work.tile([P, D], F32, name=f"aff{b}")
                    nc.vector.tensor_mul(t1[:], yn[:], lns_bc[:])
                    res = work.tile([P, D], F16, name=f"aff2_{b}")
                    nc.vector.tensor_add(res[:], t1[:], lnb_bc[:])
                nc.gpsimd.dma_start(out[:, b * D:(b + 1) * D], res[:])

    nc.compile()
    return nc


_cache = {}


def _get_program(ln_affine):
    if ln_affine not in _cache:
        _cache[ln_affine] = build_program(ln_affine=ln_affine)
    return _cache[ln_affine]


def _pack(a, blocks, row_len):
    """[blocks*128, row_len] -> [128, blocks*row_len] with logical row
    blk*128+p at (p, blk*row_len)."""
    return np.ascontiguousarray(
        a.reshape(blocks, P, row_len).transpose(1, 0, 2).reshape(
            P, blocks * row_len))


def prepare_inputs(node_features, mobility_matrix, W_in, b_in, W_out, b_out,
                   ln_scale, ln_bias):
    x = np.asarray(node_features, dtype=np.float32)
    m16 = np.asarray(mobility_matrix, dtype=np.float16)
    w_in = np.asarray(W_in, dtype=np.float64)
    b_in_ = np.asarray(b_in, dtype=np.float64)
    w_out = np.asarray(W_out, dtype=np.float64)
    b_out_ = np.asarray(b_out, dtype=np.float64)
    lns = np.asarray(ln_scale, dtype=np.float32)
    lnb = np.asarray(ln_bias, dtype=np.float32)

    w_c = (w_in @ w_out).astype(np.float32)
    bias_c = (b_in_ @ w_out + b_out_).astype(np.float32)

    s = N // NCORES
    ln_affine = not (np.all(lns == 1.0) and np.all(lnb == 0.0))

    xc = x @ w_c
    x_aug = np.zeros((N, D + 2), dtype=np.float16)
    x_aug[:, :D] = xc
    x_aug[:, D] = 1.0
    x_aug_p = _pack(x_aug, N // P, D + 2)

    in_maps = []
    for c in range(NCORES):
        msh = m16[:, c * s:(c + 1) * s]
        z1 = _pack(msh[0:S1JT * P], S1JT, s)
        z2 = np.ascontiguousarray(
            msh[S1JT * P:].reshape(Z2JT, P, s // P, P)
            .transpose(1, 2, 0, 3).reshape(P, (s // P) * Z2JT * P))
        in_maps.append({
            "m_z1": z1,
            "m_z2": z2,
            "x_aug": x_aug_p,
            "xrb": _pack(x[c * s:(c + 1) * s] + bias_c, s // P, D),
            "ln_s": lns.reshape(1, D),
            "ln_b": lnb.reshape(1, D),
        })
    return in_maps, ln_affine


def run(in_maps, ln_affine, **kwargs):
    nc = _get_program(ln_affine)
    return run_bass_kernel_spmd(nc, in_maps, core_ids=list(range(NCORES)),
                                **kwargs)


def unpack_output(res) -> np.ndarray:
    outs = []
    for c in range(NCORES):
        o = res.results[c]["out_shard"]
        outs.append(o.reshape(P, N // NCORES // P, D).transpose(1, 0, 2)
                    .reshape(N // NCORES, D).astype(np.float32))
    return np.concatenate(outs, axis=0)


def kernel(**inputs) -> np.ndarray:
    in_maps, ln_affine = prepare_inputs(**inputs)
    return unpack_output(run(in_maps, ln_affine))
